# revision 41
# baseline (speedup 1.0000x reference)
"""Trainium2 Bass kernel for single-head causal attention.

Problem: nn_Attention (dense_transformer): B=8, T=2048, C=1024, D=64, fp32.
    q = x @ Wq; k = x @ Wk; v = x @ Wv
    out = softmax(causal(q k^T / sqrt(C))) @ v

Sharding: data-parallel over batch — one batch element per NeuronCore (8 cores).
Weights replicated. Host shards/gathers; each core runs an identical program.

Per-core algorithm (f16 compute, fp32 PSUM accumulation):
  The 16 t-tiles are processed in 4 groups g of 512 t-positions. Per group:
  1. x-tiles [128,1024] f32r -> PE-transpose 128x128 blocks -> xtc [c,512t]
     f16 chunks in SBUF (PSUM->SBUF copy casts, rotated over Pool/DVE/ACT).
  2. Natural-layout fused projection: for each t-tile, lhsT = xtc chunk
     [128c,128t] (stationary), rhs = W_all [128c, 192] = [Wk|Wq|Wv] f16
     (moving): psum [128t, 192] accumulated over 8 c-chunks = [k|q|v] in
     natural [t,d] layout. One copy -> qkv_nat [128,16,193] f16 (col 192
     is a ones column making [v|1] the PV stationary, fusing the softmax
     denominator into the PV matmul).
  3. k/q re-transpose (cheap: 64-wide): per tile two PE transposes ->
     kT_sb/qT_sb [64, T] f16 at base partition 0.
  4. Attention in scoresT layout (keys on partitions), per 512-wide
     q-block qb over causal key chunks of 128:
       scoresT psum = kT-chunk.T @ qT-block      (PE, f16)
       probsT = exp(scoresT / 32) -> f16         (ACT, PSUM->SBUF)
       diagonal chunks: probsT *= causal mask    (DVE, f16 2x mode)
       poT[65,512] += [v|1]-chunk.T @ probsT     (PE, PSUM accumulate)
     Epilogue: PE-transpose poT -> [t,65]; multiply by reciprocal of the
     denominator column; DMA the q-block's output rows.
  Software pipeline: attention for q-block 3 of rep i is interleaved with
  the projection slots of rep i+1 (drained after the loop for the final
  rep), so the PE never idles at the rep boundary.
"""

import numpy as np

B, T, C, D = 8, 2048, 1024, 64
NT = T // 128       # 16 t-tiles
NC8 = C // 128      # 8 c-chunks
NG = 4              # t-groups of 4 tiles (512 t)
SCALE = 1.0 / np.sqrt(C)

_CACHE = {}


def build_nc(reps: int = 1):
    import concourse.tile as tile
    from concourse import bacc, mybir
    from concourse.masks import make_identity

    f32 = mybir.dt.float32
    f32r = mybir.dt.float32r
    f16 = mybir.dt.float16

    nc = bacc.Bacc("TRN2", target_bir_lowering=False, debug=False)
    x = nc.dram_tensor("x", [T, C], f32, kind="ExternalInput").ap()
    Wq = nc.dram_tensor("Wq", [C, D], f32, kind="ExternalInput").ap()
    Wk = nc.dram_tensor("Wk", [C, D], f32, kind="ExternalInput").ap()
    Wv = nc.dram_tensor("Wv", [C, D], f32, kind="ExternalInput").ap()
    out = nc.dram_tensor("out", [T, D], f32, kind="ExternalOutput").ap()

    xv = x.rearrange("(i p) c -> p i c", p=128).bitcast(f32r)   # [128, NT, C]
    outv = out.rearrange("(i p) d -> p i d", p=128)             # [128, NT, D]

    with tile.TileContext(nc) as tc:
        with (
            tc.tile_pool(name="const", bufs=1) as constp,
            tc.tile_pool(name="persist", bufs=1) as persist,
            tc.tile_pool(name="xg", bufs=4) as xgp,
            tc.tile_pool(name="xtc", bufs=16) as xtcp,
            tc.tile_pool(name="probs", bufs=8) as probsp,
            tc.tile_pool(name="oT", bufs=2) as oTp,
            tc.tile_pool(name="tp_ps", bufs=3, space="PSUM") as tp_ps,
            tc.tile_pool(name="qkv_ps", bufs=2, space="PSUM") as qkv_ps,
            tc.tile_pool(name="sc_ps", bufs=2, space="PSUM") as sc_ps,
            tc.tile_pool(name="po_ps", bufs=1, space="PSUM") as po_ps,
        ):
            ident = constp.tile([128, 128], f32)
            make_identity(nc, ident[:])
            ident_r = constp.tile([128, 128], f32r)
            nc.vector.tensor_copy(ident_r[:], ident[:])
            ident_h = constp.tile([128, 128], f16)
            nc.vector.tensor_copy(ident_h[:], ident[:])

            # tri[s, u] = 1.0 where s <= u else 0 (lower-causal 128x128 block)
            tri_f = constp.tile([128, 128], f32)
            nc.gpsimd.memset(tri_f[:], 1.0)
            nc.gpsimd.affine_select(
                out=tri_f[:], in_=tri_f[:],
                compare_op=mybir.AluOpType.is_ge,
                fill=0.0, base=0, channel_multiplier=-1,
                pattern=[[1, 128]],
            )
            tri = constp.tile([128, 128], f16)
            nc.vector.tensor_copy(tri[:], tri_f[:])

            # weights: [Wk | Wq | Wv] f16, chunked over c
            wstage = constp.tile([128, 3, NC8, D], f32)
            nc.sync.dma_start(wstage[:, 0], Wk.rearrange("(c8 p) j -> p c8 j", p=128))
            nc.sync.dma_start(wstage[:, 1], Wq.rearrange("(c8 p) j -> p c8 j", p=128))
            nc.sync.dma_start(wstage[:, 2], Wv.rearrange("(c8 p) j -> p c8 j", p=128))
            w_all = constp.tile([128, NC8, 3 * D], f16)
            for m in range(3):
                nc.vector.tensor_copy(w_all[:, :, D * m:D * (m + 1)],
                                      wstage[:, m])

            # persistent per-rep state
            kT_sb = persist.tile([64, T], f16)
            qT_sb = persist.tile([64, T], f16)
            qkv_nat = persist.tile([128, NT, 3 * D + 1], f16)
            out_sb = persist.tile([128, NT, D], f32)
            rcp4 = persist.tile([128, 4], f32)

            # zero-init so iteration 0's wrapped attention (garbage rep -1)
            # reads defined values; ones column for [v|1]
            nc.vector.memset(kT_sb[:], 0.0)
            nc.vector.memset(qT_sb[:], 0.0)
            nc.gpsimd.memset(qkv_nat[:], 0.0)
            nc.gpsimd.memset(qkv_nat[:, :, 3 * D], 1.0)

            def load_x(g, name):
                xg = xgp.tile([128, 4, C], f32r, tag="xg", name=name)
                src = xv[:, 4 * g:4 * g + 4, :]
                nc.sync.dma_start(xg[:, :, 0:C // 2], src[:, :, 0:C // 2])
                nc.sync.dma_start(xg[:, :, C // 2:C], src[:, :, C // 2:C])
                return xg

            # Pool/GPSIMD cannot access PSUM; rotate psum->sbuf copies DVE/ACT
            copy_engines = [
                lambda dst, src: nc.vector.tensor_copy(dst, src),
                lambda dst, src: nc.scalar.copy(dst, src),
            ]

            def front_ops(g, it_xgs, xtcs):
                """Closures for one t-group: transposes, qkv, kq-tr."""
                ops = []

                def tr(c8):
                    def f():
                        xg = it_xgs[g]
                        tp = tp_ps.tile([128, 512], f32r, tag="tp")
                        for i in range(4):
                            nc.tensor.transpose(
                                tp[:, 128 * i:128 * (i + 1)],
                                xg[:, i, 128 * c8:128 * (c8 + 1)], ident_r[:])
                        xtc = xtcp.tile([128, 512], f16, tag="xtc")
                        nc.vector.tensor_copy(xtc[:], tp[:].bitcast(f32))
                        xtcs[c8] = xtc
                    return f
                ops += [tr(c) for c in range(NC8)]

                def qkv(t):
                    def f():
                        tt = 4 * g + t
                        pq = qkv_ps.tile([128, 3 * D], f32, tag="pq")
                        for c8 in range(NC8):
                            nc.tensor.matmul(
                                pq[:], xtcs[c8][:, 128 * t:128 * (t + 1)],
                                w_all[:, c8, :],
                                start=(c8 == 0), stop=(c8 == NC8 - 1))
                        nc.scalar.copy(qkv_nat[:, tt, 0:128], pq[:, 0:128])
                        nc.vector.tensor_copy(qkv_nat[:, tt, 128:3 * D],
                                              pq[:, 128:3 * D])
                    return f
                ops += [qkv(t) for t in range(4)]

                kq_ops = []

                def kqtr(t):
                    def f():
                        tt = 4 * g + t
                        kq = tp_ps.tile([64, 256], f16, tag="tp")
                        nc.tensor.transpose(kq[:, 0:128],
                                            qkv_nat[:, tt, 0:64], ident_h[:])
                        nc.tensor.transpose(kq[:, 128:256],
                                            qkv_nat[:, tt, 64:128], ident_h[:])
                        nc.vector.tensor_copy(
                            kT_sb[:, 128 * tt:128 * (tt + 1)], kq[:, 0:128])
                        nc.vector.tensor_copy(
                            qT_sb[:, 128 * tt:128 * (tt + 1)], kq[:, 128:256])
                    return f
                kq_ops[:] = [kqtr(t) for t in range(4)]
                # interleave kq re-transposes two tiles behind the qkv mms
                ops = ops[:-2] + [kq_ops[0], ops[-2], kq_ops[1], ops[-1]] \
                    + kq_ops[2:]
                return ops

            def attn_chunk(qb, c, pos, is_first, is_last):
                """One causal key-chunk of attention for q-block qb."""
                def f():
                    j = c - 4 * qb
                    lo = 128 * j if j > 0 else 0
                    psc = sc_ps.tile([128, 512], f32, tag="sc")
                    nc.tensor.matmul(
                        psc[:, lo:512], kT_sb[:, 128 * c:128 * (c + 1)],
                        qT_sb[:, 512 * qb + lo:512 * (qb + 1)],
                        start=True, stop=True)
                    probs = probsp.tile([128, 512], f16, tag="pb")
                    nc.scalar.activation(probs[:, lo:512], psc[:, lo:512],
                                         mybir.ActivationFunctionType.Exp,
                                         scale=float(SCALE))
                    if j >= 0:
                        nc.gpsimd.tensor_mul(probs[:, lo:lo + 128],
                                             probs[:, lo:lo + 128], tri[:])
                    if is_first:
                        pos["po"] = po_ps.tile([D + 1, 512], f32, tag="po",
                                               name=f"po{qb}")
                    nc.tensor.matmul(pos["po"][:, lo:512],
                                     qkv_nat[:, c, 2 * D:3 * D + 1],
                                     probs[:, lo:512],
                                     start=is_first, stop=is_last)
                return f

            def epilogue(qb, pos):
                def fc():
                    pos["oT"] = oTp.tile([D + 1, 512], f16, tag="oT",
                                         name=f"oT{qb}")
                    for j in range(4):
                        nc.scalar.copy(pos["oT"][:, 128 * j:128 * (j + 1)],
                                       pos["po"][:, 128 * j:128 * (j + 1)])

                def f():
                    oT = pos["oT"]
                    pt = tp_ps.tile([128, 4, D + 2], f16, tag="tp",
                                    name=f"otr{qb}")
                    for j in range(4):
                        nc.tensor.transpose(pt[:, j, 0:D + 1],
                                            oT[:, 128 * j:128 * (j + 1)],
                                            ident_h[0:D + 1, 0:D + 1])
                    for j in range(4):
                        nc.vector.reciprocal(rcp4[:, j:j + 1], pt[:, j, D:D + 1])
                        nc.vector.tensor_scalar_mul(out_sb[:, 4 * qb + j, :],
                                                    pt[:, j, 0:D],
                                                    rcp4[:, j:j + 1])
                    nc.sync.dma_start(outv[:, 4 * qb:4 * qb + 4, :],
                                      out_sb[:, 4 * qb:4 * qb + 4, :])
                return [fc, f]

            def attn_ops(qb, pos, i0, i1, epi=False):
                """Chunks processed diagonal-first, full chunks last, so the
                block-end critical chain has no mask dependency."""
                order = list(range(4 * qb, 4 * (qb + 1))) + list(range(4 * qb))
                n = len(order)
                ops = [attn_chunk(qb, order[i], pos, i == 0, i == n - 1)
                       for i in range(i0, min(i1, n))]
                if epi:
                    ops.extend(epilogue(qb, pos))
                return ops

            def interleave(a, b):
                """Merge op streams evenly (b spread through a)."""
                if not b:
                    return a
                if not a:
                    return b
                out_l = []
                ratio = len(a) / len(b)
                ai = 0
                for bi, bop in enumerate(b):
                    target = int(round((bi + 1) * ratio))
                    out_l.extend(a[ai:target]); ai = target
                    out_l.append(bop)
                out_l.extend(a[ai:])
                return out_l

            def body():
                it_xgs = {}

                def dma_all():
                    for g in range(NG):
                        it_xgs[g] = load_x(g, f"xg_{g}")

                p3, p0, p1, p2 = {}, {}, {}, {}
                for g in range(NG):
                    xtcs = {}
                    front = front_ops(g, it_xgs, xtcs)
                    if g == 0:
                        # wrapped attn(3): diag chunks 12..15 then full 0..3
                        # head (covers the x DMA and precedes qkv_nat writes)
                        head = [dma_all] + attn_ops(3, p3, 0, 8)
                        ops = head + interleave(front, attn_ops(3, p3, 8, 12))
                    elif g == 1:
                        ops = interleave(front,
                                         attn_ops(3, p3, 12, 16, epi=True)
                                         + attn_ops(0, p0, 0, 4, epi=True))
                    elif g == 2:
                        ops = interleave(front,
                                         attn_ops(1, p1, 0, 8, epi=True))
                    else:
                        ops = interleave(front,
                                         attn_ops(2, p2, 0, 12, epi=True))
                    for op in ops:
                        op()

            def tail():
                p3 = {}
                for op in attn_ops(3, p3, 0, 16, epi=True):
                    op()

            UNROLL = 16
            if reps >= UNROLL:
                from concourse import mybir as _mb
                with tc.For_i(0, reps // UNROLL, 1, hint_engines=(
                        _mb.EngineType.PE, _mb.EngineType.Activation,
                        _mb.EngineType.DVE, _mb.EngineType.SP,
                        _mb.EngineType.Pool)):
                    for _ in range(UNROLL):
                        body()
            for _ in range(reps % UNROLL):
                body()
            tail()
    nc.compile()
    return nc


class _SpmdRunner:
    """Builds the jitted sharded callable once; reusable across calls."""

    def __init__(self, nc, n_cores=8):
        import jax
        import jax.numpy as jnp
        from jax.sharding import Mesh, PartitionSpec
        from jax.experimental.shard_map import shard_map
        from concourse import mybir
        from concourse.bass2jax import (_bass_exec_p, install_neuronx_cc_hook,
                                        partition_id_tensor)

        install_neuronx_cc_hook()
        self.jax = jax
        self.jnp = jnp
        self.n_cores = n_cores
        partition_name = (nc.partition_id_tensor.name
                          if nc.partition_id_tensor else None)
        in_names, out_names, out_avals, zero_outs = [], [], [], []
        for alloc in nc.m.functions[0].allocations:
            if not isinstance(alloc, mybir.MemoryLocationSet):
                continue
            name = alloc.memorylocations[0].name
            if alloc.kind == "ExternalInput":
                if name != partition_name:
                    in_names.append(name)
            elif alloc.kind == "ExternalOutput":
                out_names.append(name)
                shape = tuple(alloc.tensor_shape)
                dtype = mybir.dt.np(alloc.dtype)
                out_avals.append(jax.core.ShapedArray(shape, dtype))
                zero_outs.append((shape, dtype))
        self.in_names, self.out_names = in_names, out_names
        self.out_avals, self.zero_outs = out_avals, zero_outs
        n_params = len(in_names)
        self.n_params = n_params
        all_in_names = list(in_names) + list(out_names)
        if partition_name is not None:
            all_in_names.append(partition_name)

        def _body(*args):
            operands = list(args)
            if partition_name is not None:
                operands.append(partition_id_tensor())
            outs = _bass_exec_p.bind(
                *operands,
                out_avals=tuple(out_avals),
                in_names=tuple(all_in_names),
                out_names=tuple(out_names),
                lowering_input_output_aliases=(),
                sim_require_finite=True,
                sim_require_nnan=True,
                nc=nc,
            )
            return tuple(outs)

        devices = jax.devices()[:n_cores]
        mesh = Mesh(np.asarray(devices), ("core",))
        n_outs = len(out_names)
        in_specs = (PartitionSpec("core"),) * (n_params + n_outs)
        out_specs = (PartitionSpec("core"),) * n_outs
        donate = tuple(range(n_params, n_params + n_outs))
        self.sharded = jax.jit(
            shard_map(_body, mesh=mesh, in_specs=in_specs,
                      out_specs=out_specs, check_rep=False),
            donate_argnums=donate, keep_unused=True)
        self._zeros_fn = jax.jit(
            lambda: tuple(jnp.zeros((n_cores * s[0], *s[1:]), d)
                          for (s, d) in zero_outs))

    def put_inputs(self, in_maps):
        per_core = [[np.asarray(m[n]) for n in self.in_names] for m in in_maps]
        concat = [np.concatenate([per_core[c][i] for c in range(self.n_cores)], axis=0)
                  for i in range(self.n_params)]
        return [self.jax.device_put(a) for a in concat]

    def make_zeros_dev(self):
        z = self._zeros_fn()
        self.jax.block_until_ready(z)
        return list(z)

    def run(self, dev_in, zeros=None):
        if zeros is None:
            zeros = self.make_zeros_dev()
        outs = self.sharded(*dev_in, *zeros)
        self.jax.block_until_ready(outs)
        return outs

    def gather(self, outs):
        return [
            {n: np.asarray(outs[i]).reshape(self.n_cores, *self.out_avals[i].shape)[c]
             for i, n in enumerate(self.out_names)}
            for c in range(self.n_cores)
        ]


def _get_runner():
    if "runner" not in _CACHE:
        _CACHE["runner"] = _SpmdRunner(build_nc(reps=1), n_cores=B)
    return _CACHE["runner"]


def kernel(x, Wq, Wk, Wv):
    x = np.ascontiguousarray(np.asarray(x, dtype=np.float32))
    Wq = np.ascontiguousarray(np.asarray(Wq, dtype=np.float32))
    Wk = np.ascontiguousarray(np.asarray(Wk, dtype=np.float32))
    Wv = np.ascontiguousarray(np.asarray(Wv, dtype=np.float32))
    runner = _get_runner()
    in_maps = [{"x": x[b], "Wq": Wq, "Wk": Wk, "Wv": Wv} for b in range(B)]
    dev_in = runner.put_inputs(in_maps)
    res = runner.gather(runner.run(dev_in))
    return np.stack([res[b]["out"] for b in range(B)], axis=0)
